# revision 7
# baseline (speedup 1.0000x reference)
"""Causal attention (LN -> QKV -> 16-head causal attn -> out-proj) on 8 TRN2 cores.

Sharding: core c = (batch b=c//4, head-group g=c%4). Each core runs its batch's
LayerNorm + a 4-head slice of QKV / attention / out-projection. The out-proj
partials (column-split over the inner dim) are summed on the host per batch.

v2 layout notes (per core):
  - Weights are pre-cast to bf16 and pre-permuted on the HOST so device DMAs
    are contiguous 128x4KB loads and no on-device casts are needed.
  - xn transpose: XBAR dma_start_transpose (SBUF->SBUF, bf16) writes
    xnT[p, sbq, kb, s] = xn[s, p*KB + kb]; weights use the matching
    d = p*KB + kb row permutation (w.reshape(128, KB, M) on host).
  - Attention inner loop is software-pipelined: PE issue order is
    S^T(kb,h0), PV(kb-1,h0), S^T(kb,h1), PV(kb-1,h1) so PE never idles
    waiting for exp (ScalarE) in steady state. S^T matmuls are trimmed at
    the causal boundary.
  - V tiles carry a 65th all-ones column so PV psum row 64 accumulates the
    softmax denominators. Normalization is fused into the PV evacuation:
    recip of the denom row (DVE), partition-broadcast via DMA, then
    tensor_tensor(outT, ps_o, recip_bc, mult).
"""

import numpy as np
import ml_dtypes

import concourse.bass as bass
import concourse.mybir as mybir
import concourse.tile as tile
from concourse import bacc
from concourse.bass_utils import run_bass_kernel_spmd
from concourse.masks import make_identity

B, N, DIM, HEADS, DIM_HEAD = 2, 2048, 1024, 16, 64
INNER = HEADS * DIM_HEAD
H_LOC = 4                      # heads per core
N_CORES = 8
P = 128
NB = N // P                    # 16 seq blocks
KB = DIM // P                  # 8 dim blocks
QT = 512                       # psum-bank-sized q tile
HALF = 1024                    # q span per S^T psum tile
SCALE = DIM_HEAD ** -0.5
LN_EPS = 1e-5

F32 = mybir.dt.float32
BF16 = mybir.dt.bfloat16
AF = mybir.ActivationFunctionType
ALU = mybir.AluOpType

USE_DMA_T = True               # XBAR dma transpose for xn (else PE transpose)
BCAST_SBUF = False              # SBUF->SBUF broadcast DMA (else DRAM hop)


def build_nc():
    from contextlib import ExitStack

    nc = bacc.Bacc(None, target_bir_lowering=False, debug=False)

    x_d = nc.dram_tensor("x", [N, DIM], F32, kind="ExternalInput")
    wq_d = nc.dram_tensor("wq", [P, KB, H_LOC * DIM_HEAD], BF16, kind="ExternalInput")
    wk_d = nc.dram_tensor("wk", [P, KB, H_LOC * DIM_HEAD], BF16, kind="ExternalInput")
    wv_d = nc.dram_tensor("wv", [P, KB, H_LOC * DIM_HEAD], BF16, kind="ExternalInput")
    wo_d = nc.dram_tensor("wo", [P, 2, DIM], BF16, kind="ExternalInput")
    bq_d = nc.dram_tensor("bq", [P, 2], F32, kind="ExternalInput")
    bk_d = nc.dram_tensor("bk", [P, 2], F32, kind="ExternalInput")
    bv_d = nc.dram_tensor("bv", [1, H_LOC * DIM_HEAD], F32, kind="ExternalInput")
    out_d = nc.dram_tensor("out", [N, DIM], F32, kind="ExternalOutput")

    with tile.TileContext(nc) as tc:
        ctx = ExitStack()
        with ctx:
            const = ctx.enter_context(tc.tile_pool(name="const", bufs=1))
            persist = ctx.enter_context(tc.tile_pool(name="persist", bufs=1))
            xpool = ctx.enter_context(tc.tile_pool(name="xpool", bufs=5))
            xnpool = ctx.enter_context(tc.tile_pool(name="xnpool", bufs=4))
            stat = ctx.enter_context(tc.tile_pool(name="stat", bufs=8))
            expp = ctx.enter_context(tc.tile_pool(name="expp", bufs=3))
            drp = ctx.enter_context(tc.tile_pool(name="drp", bufs=4))
            rbcp = ctx.enter_context(tc.tile_pool(name="rbcp", bufs=4))
            dramp = ctx.enter_context(tc.tile_pool(name="dramp", bufs=4, space="DRAM"))
            stage = ctx.enter_context(tc.tile_pool(name="stage", bufs=3))

            # ---- first x blocks before the weight loads: LN of sb0 starts
            # as soon as 512KB lands instead of queueing behind the weights.
            x_ts = {}
            for sb in range(3):
                x_ts[sb] = xpool.tile([P, DIM], F32, tag="x", name=f"x{sb}")
                nc.sync.dma_start(x_ts[sb][:], x_d[sb * P:(sb + 1) * P, :])

            # ---- constants / weights (bf16, host-permuted, contiguous) ----
            eps_t = const.tile([P, 1], F32, tag="eps")
            nc.vector.memset(eps_t, LN_EPS)
            bq_sb = const.tile([P, 2], F32, tag="bq")
            nc.sync.dma_start(bq_sb[:], bq_d[:])
            bk_sb = const.tile([P, 2], F32, tag="bk")
            nc.sync.dma_start(bk_sb[:], bk_d[:])
            bv_sb = const.tile([P, H_LOC, DIM_HEAD], F32, tag="bv")
            nc.sync.dma_start(
                bv_sb[:],
                bv_d[:].rearrange("o (h d) -> o h d", h=H_LOC)
                .to_broadcast((P, H_LOC, DIM_HEAD)),
            )
            wv_bf = persist.tile([P, KB, H_LOC * DIM_HEAD], BF16, tag="wv")
            nc.sync.dma_start(wv_bf[:], wv_d[:])
            wq_bf = persist.tile([P, KB, H_LOC * DIM_HEAD], BF16, tag="wq")
            nc.sync.dma_start(wq_bf[:], wq_d[:])
            wk_bf = persist.tile([P, KB, H_LOC * DIM_HEAD], BF16, tag="wk")
            nc.sync.dma_start(wk_bf[:], wk_d[:])
            wo_bf = persist.tile([P, 2, DIM], BF16, tag="wo")
            nc.sync.dma_start(wo_bf[:], wo_d[:])

            if not USE_DMA_T:
                ident = const.tile([P, P], BF16, tag="ident")
                make_identity(nc, ident)
            # keep-mask for the causal diagonal block: tri[k, q] = (k <= q)
            tri = const.tile([P, P], BF16, tag="tri")
            nc.gpsimd.memset(tri[:], 0.0)
            nc.gpsimd.affine_select(
                out=tri[:], in_=tri[:], compare_op=ALU.is_gt, fill=1.0,
                base=0, channel_multiplier=1, pattern=[[-1, P]],
            )

            # xnT quarters: [p, sbq, kb, s]; xnT[q][p, j, kb, s] = xn[(4q+j)*P+s, p*KB+kb]
            # (DMA-T layout; PE-transpose fallback uses d = kb*P + p and the
            #  host permutation matches via PERM_PKB flag in make_in_maps.)
            xnT = [persist.tile([P, 4, KB, P], BF16, tag=f"xnT{q}", name=f"xnT{q}")
                   for q in range(4)]
            QTt = [persist.tile([P, N], BF16, tag=f"qt{p_}", name=f"qt{p_}")
                   for p_ in range(2)]
            KTt = [persist.tile([P, N], BF16, tag=f"kt{p_}", name=f"kt{p_}")
                   for p_ in range(2)]
            Vt = persist.tile([P, NB, H_LOC, DIM_HEAD + 1], BF16, tag="v")
            nc.gpsimd.memset(Vt[:], 1.0)  # 65th column stays 1.0 -> denominators
            outT = [[persist.tile([P, HALF], BF16, tag=f"outT{p_}_{q_}",
                                  name=f"outT{p_}_{q_}") for q_ in range(2)]
                    for p_ in range(2)]

            # ---- phase A: LN -> transpose -> QKV -> V (interleaved) ----
            psA_cm = tc.tile_pool(name="psA", bufs=4, space="PSUM")
            psA = psA_cm.__enter__()

            def emit_qkv_st(st):
                for (wt, bias_sb, dstt) in ((wq_bf, bq_sb, QTt), (wk_bf, bk_sb, KTt)):
                    for pr in range(2):
                        ps = psA.tile([P, 512], F32, tag="ps")
                        for kb in range(KB):
                            nc.tensor.matmul(
                                ps[:],
                                wt[:, kb, pr * P:(pr + 1) * P],
                                xnT[st][:, :, kb, :],
                                start=(kb == 0), stop=(kb == KB - 1),
                            )
                        # bias-add evacuation on ScalarE (per-partition bias)
                        nc.scalar.activation(
                            dstt[pr][:, st * 512:(st + 1) * 512], ps[:],
                            AF.Identity, bias=bias_sb[:, pr:pr + 1],
                        )

            for sb in range(NB):
                if sb + 3 < NB:
                    x_ts[sb + 3] = xpool.tile([P, DIM], F32, tag="x",
                                              name=f"x{sb + 3}")
                    nc.sync.dma_start(x_ts[sb + 3][:],
                                      x_d[(sb + 3) * P:(sb + 4) * P, :])
                x_t = x_ts.pop(sb)

                stats = stat.tile([P, 2, 6], F32, tag="bnst")
                x3 = x_t[:].rearrange("p (a f) -> p a f", a=2)
                for a in range(2):
                    nc.vector.bn_stats(stats[:, a, :], x3[:, a, :])
                mv = stat.tile([P, 2], F32, tag="mv")
                nc.vector.bn_aggr(mv[:], stats[:])
                rstd = stat.tile([P, 1], F32, tag="rstd")
                nc.scalar.activation(rstd[:], mv[:, 1:2], AF.Sqrt, bias=eps_t[:])
                nc.vector.reciprocal(rstd[:], rstd[:])
                # nmrs = -mean * rstd  -> xn = x*rstd + nmrs on ScalarE
                nmrs = stat.tile([P, 1], F32, tag="nmrs")
                nc.vector.tensor_scalar(
                    nmrs[:], mv[:, 0:1], rstd[:], -1.0, ALU.mult, ALU.mult
                )
                xn_bf = xnpool.tile([P, DIM], BF16, tag="xn")
                nc.scalar.activation(
                    xn_bf[:], x_t[:], AF.Identity, bias=nmrs[:], scale=rstd[:]
                )

                # transpose this seq block into xnT[sb//4][:, sb%4, :, :]
                # (two half-transposes land on different DMA queues: halves
                #  the ~10us single-queue latency of a 256KB XBAR transfer)
                if USE_DMA_T:
                    for half in range(2):
                        nc.sync.dma_start_transpose(
                            xnT[sb // 4][:, sb % 4, half * 4:(half + 1) * 4, :],
                            xn_bf[:, half * 512:(half + 1) * 512],
                        )
                else:
                    for half in range(2):
                        ps = psA.tile([P, 512], F32, tag="ps")
                        for j in range(4):
                            kb = half * 4 + j
                            nc.tensor.matmul(
                                ps[:, j * P:(j + 1) * P],
                                xn_bf[:, kb * P:(kb + 1) * P],
                                ident[:],
                                start=True, stop=True,
                            )
                        dst = xnT[sb // 4][:, sb % 4, half * 4:(half + 1) * 4, :]
                        src = ps[:].rearrange("p (a f) -> p a f", a=4)
                        if half == 0:
                            nc.scalar.copy(dst, src)
                        else:
                            nc.vector.tensor_copy(dst, src)

                # V for this seq block
                ps = psA.tile([P, 512], F32, tag="ps")
                psv = ps[:, :H_LOC * DIM_HEAD]
                for kb in range(KB):
                    nc.tensor.matmul(
                        psv,
                        xnT[sb // 4][:, sb % 4, kb, :],
                        wv_bf[:, kb, :],
                        start=(kb == 0), stop=(kb == KB - 1),
                    )
                nc.vector.tensor_tensor(
                    Vt[:, sb, :, :DIM_HEAD],
                    psv.rearrange("p (h d) -> p h d", h=H_LOC),
                    bv_sb[:],
                    ALU.add,
                )

                if sb % 4 == 3:
                    emit_qkv_st(sb // 4)

            psA_cm.__exit__(None, None, None)

            # ---- phase B: attention, software-pipelined over key blocks ----
            ctx2 = ExitStack()
            with ctx2:
                psS = ctx2.enter_context(tc.tile_pool(name="psS", bufs=1, space="PSUM"))
                psO = ctx2.enter_context(tc.tile_pool(name="psO", bufs=1, space="PSUM"))

                for qh in range(2):
                    qs, qe = qh * HALF, (qh + 1) * HALF
                    for pr in range(2):
                        ps_o = [psO.tile([DIM_HEAD + 1, HALF], F32,
                                         tag=f"po{hh}", name=f"po{hh}_{pr}_{qh}")
                                for hh in range(2)]
                        last_kb = qe // P - 1

                        def emit_pv(kb, ex, hh):
                            qlo = kb * P
                            for qt in range(qs // QT, qe // QT):
                                rs, re = qt * QT, (qt + 1) * QT
                                if re <= qlo:
                                    continue
                                cs = max(qlo, rs)
                                nc.tensor.matmul(
                                    ps_o[hh][:, cs - qs:re - qs],
                                    Vt[:, kb, 2 * pr + hh, :],
                                    ex[:, hh, cs - qs:re - qs],
                                    start=(kb == 0),
                                    stop=(kb == min(last_kb, re // P - 1)),
                                )

                        prev = None  # (kb, ex) with PV not yet emitted
                        for kb in range(qe // P):
                            qlo = kb * P
                            ex = expp.tile([P, 2, HALF], BF16, tag="ex",
                                           name=f"ex_{pr}_{qh}_{kb}")
                            vstart = max(qlo, qs)
                            for hh in range(2):
                                s_ps = psS.tile([P, HALF], F32, tag=f"s{hh}",
                                                name=f"s{hh}_{pr}_{qh}_{kb}")
                                po = hh * DIM_HEAD
                                for qt in range(qs // QT, qe // QT):
                                    rs, re = qt * QT, (qt + 1) * QT
                                    if re <= qlo:
                                        continue
                                    cs = max(qlo, rs)
                                    nc.tensor.matmul(
                                        s_ps[:, cs - qs:re - qs],
                                        KTt[pr][po:po + DIM_HEAD, qlo:qlo + P],
                                        QTt[pr][po:po + DIM_HEAD, cs:re],
                                        start=True, stop=True,
                                        tile_position=(po, 0),
                                    )
                                nc.scalar.activation(
                                    ex[:, hh, vstart - qs:],
                                    s_ps[:, vstart - qs:],
                                    AF.Exp,
                                )
                                if qlo >= qs:
                                    nc.vector.tensor_tensor(
                                        ex[:, hh, qlo - qs:qlo - qs + P],
                                        ex[:, hh, qlo - qs:qlo - qs + P],
                                        tri[:],
                                        ALU.mult,
                                    )
                                if prev is not None:
                                    emit_pv(prev[0], prev[1], hh)
                            prev = (kb, ex)
                        for hh in range(2):
                            emit_pv(prev[0], prev[1], hh)

                        # evacuate unnormalized output + denom rows quickly
                        # (releases the PV psum banks for the next head pair),
                        # then normalize outT in place: reciprocal runs in a
                        # [128, 16] layout via DRAM shuffles (partition-1
                        # tiles pay full free-size cycles on DVE).
                        da = dramp.tile([2, HALF], F32, tag="da",
                                        name=f"da{pr}_{qh}")
                        for hh in range(2):
                            nc.vector.tensor_copy(
                                outT[pr][qh][hh * DIM_HEAD:(hh + 1) * DIM_HEAD, :],
                                ps_o[hh][:DIM_HEAD, :],
                            )
                            dr = drp.tile([1, HALF], F32, tag=f"dr{hh}",
                                          name=f"dr{pr}_{qh}_{hh}")
                            nc.scalar.copy(
                                dr[:], ps_o[hh][DIM_HEAD:DIM_HEAD + 1, :]
                            )
                            nc.sync.dma_start(da[hh:hh + 1, :], dr[:])
                        dsh = drp.tile([P, 2, HALF // P], F32, tag="dsh",
                                       name=f"dsh{pr}_{qh}")
                        nc.sync.dma_start(
                            dsh[:],
                            da[:].rearrange("h (p o) -> p h o", o=HALF // P),
                        )
                        nc.vector.reciprocal(dsh[:], dsh[:])
                        db = dramp.tile([2, HALF], F32, tag="db",
                                        name=f"db{pr}_{qh}")
                        nc.sync.dma_start(
                            db[:].rearrange("h (p o) -> p h o", o=HALF // P),
                            dsh[:],
                        )
                        rbc = rbcp.tile([P, HALF], F32, tag="rbc",
                                        name=f"rbc{pr}_{qh}")
                        for hh in range(2):
                            nc.sync.dma_start(
                                rbc[hh * DIM_HEAD:(hh + 1) * DIM_HEAD, :],
                                db[hh:hh + 1, :].to_broadcast((DIM_HEAD, HALF)),
                            )
                        nc.vector.tensor_tensor(
                            outT[pr][qh][:], outT[pr][qh][:], rbc[:], ALU.mult
                        )

            # ---- phase C: out projection ----
            psP = ctx.enter_context(tc.tile_pool(name="psP", bufs=3, space="PSUM"))
            for qb in range(NB):
                ps = psP.tile([P, 2, 512], F32, tag="pp")
                for nt in range(2):
                    for pb in range(2):
                        nc.tensor.matmul(
                            ps[:, nt, :],
                            outT[pb][qb // 8][:, (qb % 8) * P:(qb % 8 + 1) * P],
                            wo_bf[:, pb, nt * 512:(nt + 1) * 512],
                            start=(pb == 0), stop=(pb == 1),
                        )
                so = stage.tile([P, DIM], F32, tag="so")
                if qb % 2 == 0:
                    nc.scalar.copy(so[:], ps[:].rearrange("p a f -> p (a f)"))
                else:
                    nc.vector.tensor_copy(so[:], ps[:].rearrange("p a f -> p (a f)"))
                nc.sync.dma_start(out_d[qb * P:(qb + 1) * P, :], so[:])

    nc.compile()
    return nc


def make_in_maps(x, ln_w, ln_b, w_qkv, w_out):
    x = np.asarray(x, np.float32)
    ln_w = np.asarray(ln_w, np.float32)
    ln_b = np.asarray(ln_b, np.float32)
    w_qkv = np.asarray(w_qkv, np.float32)
    w_out = np.asarray(w_out, np.float32)
    bf16 = ml_dtypes.bfloat16

    def perm_w(w):
        # device row (p, kb) holds dim d = kb*P + p — both for the XBAR DMA
        # transpose (verified on hw) and the PE-transpose fallback
        return np.ascontiguousarray(
            w.reshape(KB, P, -1).transpose(1, 0, 2)).astype(bf16)

    in_maps = []
    for c in range(N_CORES):
        b, g = c // 4, c % 4
        cols = np.arange(4 * g * DIM_HEAD, (4 * g + H_LOC) * DIM_HEAD)
        wq_s = w_qkv[:, cols]
        wk_s = w_qkv[:, INNER + cols]
        wv_s = w_qkv[:, 2 * INNER + cols]
        wq = perm_w(ln_w[:, None] * wq_s * SCALE)
        wk = perm_w(ln_w[:, None] * wk_s)
        wv = perm_w(ln_w[:, None] * wv_s)
        wo = np.ascontiguousarray(
            w_out[cols, :].reshape(2, P, DIM).transpose(1, 0, 2)).astype(bf16)
        bq = (ln_b @ wq_s) * SCALE
        bk = ln_b @ wk_s
        bv = ln_b @ wv_s
        in_maps.append({
            "x": np.ascontiguousarray(x[b]),
            "wq": wq, "wk": wk, "wv": wv, "wo": wo,
            "bq": np.ascontiguousarray(bq.reshape(2, P).T),
            "bk": np.ascontiguousarray(bk.reshape(2, P).T),
            "bv": bv.reshape(1, H_LOC * DIM_HEAD),
        })
    return in_maps


_NC_CACHE = []


def kernel(x, ln_w, ln_b, w_qkv, w_out):
    in_maps = make_in_maps(x, ln_w, ln_b, w_qkv, w_out)
    if not _NC_CACHE:
        _NC_CACHE.append(build_nc())
    nc = _NC_CACHE[0]
    res = run_bass_kernel_spmd(nc, in_maps, list(range(N_CORES))).results
    out = np.zeros((B, N, DIM), np.float32)
    for c in range(N_CORES):
        out[c // 4] += res[c]["out"]
    return out


# revision 8
# speedup vs baseline: 1.2399x; 1.2399x over previous
"""Causal attention (LN -> QKV -> 16-head causal attn -> out-proj) on 8 TRN2 cores.

Sharding: core c = (batch b=c//4, head-group g=c%4). Each core runs its batch's
LayerNorm + a 4-head slice of QKV / attention / out-projection. The out-proj
partials (column-split over the inner dim) are summed on the host per batch.

v2 layout notes (per core):
  - Weights are pre-cast to bf16 and pre-permuted on the HOST so device DMAs
    are contiguous 128x4KB loads and no on-device casts are needed.
  - xn transpose: XBAR dma_start_transpose (SBUF->SBUF, bf16) writes
    xnT[p, sbq, kb, s] = xn[s, p*KB + kb]; weights use the matching
    d = p*KB + kb row permutation (w.reshape(128, KB, M) on host).
  - Attention inner loop is software-pipelined: PE issue order is
    S^T(kb,h0), PV(kb-1,h0), S^T(kb,h1), PV(kb-1,h1) so PE never idles
    waiting for exp (ScalarE) in steady state. S^T matmuls are trimmed at
    the causal boundary.
  - V tiles carry a 65th all-ones column so PV psum row 64 accumulates the
    softmax denominators. Normalization is fused into the PV evacuation:
    recip of the denom row (DVE), partition-broadcast via DMA, then
    tensor_tensor(outT, ps_o, recip_bc, mult).
"""

import numpy as np
import ml_dtypes

import concourse.bass as bass
import concourse.mybir as mybir
import concourse.tile as tile
from concourse import bacc
from concourse.bass_utils import run_bass_kernel_spmd
from concourse.masks import make_identity

B, N, DIM, HEADS, DIM_HEAD = 2, 2048, 1024, 16, 64
INNER = HEADS * DIM_HEAD
H_LOC = 4                      # heads per core
N_CORES = 8
P = 128
NB = N // P                    # 16 seq blocks
KB = DIM // P                  # 8 dim blocks
QT = 512                       # psum-bank-sized q tile
HALF = 1024                    # q span per S^T psum tile
SCALE = DIM_HEAD ** -0.5
LN_EPS = 1e-5

F32 = mybir.dt.float32
BF16 = mybir.dt.bfloat16
AF = mybir.ActivationFunctionType
ALU = mybir.AluOpType

USE_DMA_T = False               # XBAR dma transpose for xn (else PE transpose)
BCAST_SBUF = False              # SBUF->SBUF broadcast DMA (else DRAM hop)


def build_nc():
    from contextlib import ExitStack

    nc = bacc.Bacc(None, target_bir_lowering=False, debug=False)

    x_d = nc.dram_tensor("x", [N, DIM], F32, kind="ExternalInput")
    wq_d = nc.dram_tensor("wq", [P, KB, H_LOC * DIM_HEAD], BF16, kind="ExternalInput")
    wk_d = nc.dram_tensor("wk", [P, KB, H_LOC * DIM_HEAD], BF16, kind="ExternalInput")
    wv_d = nc.dram_tensor("wv", [P, KB, H_LOC * DIM_HEAD], BF16, kind="ExternalInput")
    wo_d = nc.dram_tensor("wo", [P, 2, DIM], BF16, kind="ExternalInput")
    bq_d = nc.dram_tensor("bq", [P, 2], F32, kind="ExternalInput")
    bk_d = nc.dram_tensor("bk", [P, 2], F32, kind="ExternalInput")
    bv_d = nc.dram_tensor("bv", [1, H_LOC * DIM_HEAD], F32, kind="ExternalInput")
    out_d = nc.dram_tensor("out", [N, DIM], F32, kind="ExternalOutput")

    with tile.TileContext(nc) as tc:
        ctx = ExitStack()
        with ctx:
            const = ctx.enter_context(tc.tile_pool(name="const", bufs=1))
            persist = ctx.enter_context(tc.tile_pool(name="persist", bufs=1))
            xpool = ctx.enter_context(tc.tile_pool(name="xpool", bufs=5))
            xnpool = ctx.enter_context(tc.tile_pool(name="xnpool", bufs=4))
            stat = ctx.enter_context(tc.tile_pool(name="stat", bufs=8))
            expp = ctx.enter_context(tc.tile_pool(name="expp", bufs=3))
            drp = ctx.enter_context(tc.tile_pool(name="drp", bufs=4))
            rbcp = ctx.enter_context(tc.tile_pool(name="rbcp", bufs=4))
            dramp = ctx.enter_context(tc.tile_pool(name="dramp", bufs=4, space="DRAM"))
            stage = ctx.enter_context(tc.tile_pool(name="stage", bufs=3))

            # ---- first x blocks before the weight loads: LN of sb0 starts
            # as soon as 512KB lands instead of queueing behind the weights.
            x_ts = {}
            for sb in range(3):
                x_ts[sb] = xpool.tile([P, DIM], F32, tag="x", name=f"x{sb}")
                nc.sync.dma_start(x_ts[sb][:], x_d[sb * P:(sb + 1) * P, :])

            # ---- constants / weights (bf16, host-permuted, contiguous) ----
            eps_t = const.tile([P, 1], F32, tag="eps")
            nc.vector.memset(eps_t, LN_EPS)
            bq_sb = const.tile([P, 2], F32, tag="bq")
            nc.sync.dma_start(bq_sb[:], bq_d[:])
            bk_sb = const.tile([P, 2], F32, tag="bk")
            nc.sync.dma_start(bk_sb[:], bk_d[:])
            bv_sb = const.tile([P, H_LOC, DIM_HEAD], F32, tag="bv")
            nc.sync.dma_start(
                bv_sb[:],
                bv_d[:].rearrange("o (h d) -> o h d", h=H_LOC)
                .to_broadcast((P, H_LOC, DIM_HEAD)),
            )
            wv_bf = persist.tile([P, KB, H_LOC * DIM_HEAD], BF16, tag="wv")
            nc.sync.dma_start(wv_bf[:], wv_d[:])
            wq_bf = persist.tile([P, KB, H_LOC * DIM_HEAD], BF16, tag="wq")
            nc.sync.dma_start(wq_bf[:], wq_d[:])
            wk_bf = persist.tile([P, KB, H_LOC * DIM_HEAD], BF16, tag="wk")
            nc.sync.dma_start(wk_bf[:], wk_d[:])
            wo_bf = persist.tile([P, 2, DIM], BF16, tag="wo")
            nc.sync.dma_start(wo_bf[:], wo_d[:])

            if not USE_DMA_T:
                ident = const.tile([P, P], BF16, tag="ident")
                make_identity(nc, ident)
            # keep-mask for the causal diagonal block: tri[k, q] = (k <= q)
            tri = const.tile([P, P], BF16, tag="tri")
            nc.gpsimd.memset(tri[:], 0.0)
            nc.gpsimd.affine_select(
                out=tri[:], in_=tri[:], compare_op=ALU.is_gt, fill=1.0,
                base=0, channel_multiplier=1, pattern=[[-1, P]],
            )

            # xnT quarters: [p, sbq, kb, s]; xnT[q][p, j, kb, s] = xn[(4q+j)*P+s, p*KB+kb]
            # (DMA-T layout; PE-transpose fallback uses d = kb*P + p and the
            #  host permutation matches via PERM_PKB flag in make_in_maps.)
            xnT = [persist.tile([P, 4, KB, P], BF16, tag=f"xnT{q}", name=f"xnT{q}")
                   for q in range(4)]
            QTt = [persist.tile([P, N], BF16, tag=f"qt{p_}", name=f"qt{p_}")
                   for p_ in range(2)]
            KTt = [persist.tile([P, N], BF16, tag=f"kt{p_}", name=f"kt{p_}")
                   for p_ in range(2)]
            Vt = persist.tile([P, NB, H_LOC, DIM_HEAD + 1], BF16, tag="v")
            nc.gpsimd.memset(Vt[:], 1.0)  # 65th column stays 1.0 -> denominators
            outT = [[persist.tile([P, HALF], BF16, tag=f"outT{p_}_{q_}",
                                  name=f"outT{p_}_{q_}") for q_ in range(2)]
                    for p_ in range(2)]

            # ---- phase A: LN -> transpose -> QKV -> V (interleaved) ----
            psA_cm = tc.tile_pool(name="psA", bufs=4, space="PSUM")
            psA = psA_cm.__enter__()

            def emit_qkv_st(st):
                for (wt, bias_sb, dstt) in ((wq_bf, bq_sb, QTt), (wk_bf, bk_sb, KTt)):
                    for pr in range(2):
                        ps = psA.tile([P, 512], F32, tag="ps")
                        for kb in range(KB):
                            nc.tensor.matmul(
                                ps[:],
                                wt[:, kb, pr * P:(pr + 1) * P],
                                xnT[st][:, :, kb, :],
                                start=(kb == 0), stop=(kb == KB - 1),
                            )
                        # bias-add evacuation on ScalarE (per-partition bias)
                        nc.scalar.activation(
                            dstt[pr][:, st * 512:(st + 1) * 512], ps[:],
                            AF.Identity, bias=bias_sb[:, pr:pr + 1],
                        )

            for sb in range(NB):
                if sb + 3 < NB:
                    x_ts[sb + 3] = xpool.tile([P, DIM], F32, tag="x",
                                              name=f"x{sb + 3}")
                    nc.sync.dma_start(x_ts[sb + 3][:],
                                      x_d[(sb + 3) * P:(sb + 4) * P, :])
                x_t = x_ts.pop(sb)

                stats = stat.tile([P, 2, 6], F32, tag="bnst")
                x3 = x_t[:].rearrange("p (a f) -> p a f", a=2)
                for a in range(2):
                    nc.vector.bn_stats(stats[:, a, :], x3[:, a, :])
                mv = stat.tile([P, 2], F32, tag="mv")
                nc.vector.bn_aggr(mv[:], stats[:])
                rstd = stat.tile([P, 1], F32, tag="rstd")
                nc.scalar.activation(rstd[:], mv[:, 1:2], AF.Sqrt, bias=eps_t[:])
                nc.vector.reciprocal(rstd[:], rstd[:])
                # nmrs = -mean * rstd  -> xn = x*rstd + nmrs on ScalarE
                nmrs = stat.tile([P, 1], F32, tag="nmrs")
                nc.vector.tensor_scalar(
                    nmrs[:], mv[:, 0:1], rstd[:], -1.0, ALU.mult, ALU.mult
                )
                xn_bf = xnpool.tile([P, DIM], BF16, tag="xn")
                nc.scalar.activation(
                    xn_bf[:], x_t[:], AF.Identity, bias=nmrs[:], scale=rstd[:]
                )

                # transpose this seq block into xnT[sb//4][:, sb%4, :, :]
                # (two half-transposes land on different DMA queues: halves
                #  the ~10us single-queue latency of a 256KB XBAR transfer)
                if USE_DMA_T:
                    for half in range(2):
                        nc.sync.dma_start_transpose(
                            xnT[sb // 4][:, sb % 4, half * 4:(half + 1) * 4, :],
                            xn_bf[:, half * 512:(half + 1) * 512],
                        )
                else:
                    for half in range(2):
                        ps = psA.tile([P, 512], F32, tag="ps")
                        for j in range(4):
                            kb = half * 4 + j
                            nc.tensor.matmul(
                                ps[:, j * P:(j + 1) * P],
                                xn_bf[:, kb * P:(kb + 1) * P],
                                ident[:],
                                start=True, stop=True,
                            )
                        dst = xnT[sb // 4][:, sb % 4, half * 4:(half + 1) * 4, :]
                        src = ps[:].rearrange("p (a f) -> p a f", a=4)
                        if half == 0:
                            nc.scalar.copy(dst, src)
                        else:
                            nc.vector.tensor_copy(dst, src)

                # V for this seq block
                ps = psA.tile([P, 512], F32, tag="ps")
                psv = ps[:, :H_LOC * DIM_HEAD]
                for kb in range(KB):
                    nc.tensor.matmul(
                        psv,
                        xnT[sb // 4][:, sb % 4, kb, :],
                        wv_bf[:, kb, :],
                        start=(kb == 0), stop=(kb == KB - 1),
                    )
                nc.vector.tensor_tensor(
                    Vt[:, sb, :, :DIM_HEAD],
                    psv.rearrange("p (h d) -> p h d", h=H_LOC),
                    bv_sb[:],
                    ALU.add,
                )

                if sb % 4 == 3:
                    emit_qkv_st(sb // 4)

            psA_cm.__exit__(None, None, None)

            # ---- phase B: attention, software-pipelined over key blocks ----
            ctx2 = ExitStack()
            with ctx2:
                psS = ctx2.enter_context(tc.tile_pool(name="psS", bufs=1, space="PSUM"))
                psO = ctx2.enter_context(tc.tile_pool(name="psO", bufs=1, space="PSUM"))

                for qh in range(2):
                    qs, qe = qh * HALF, (qh + 1) * HALF
                    for pr in range(2):
                        ps_o = [psO.tile([DIM_HEAD + 1, HALF], F32,
                                         tag=f"po{hh}", name=f"po{hh}_{pr}_{qh}")
                                for hh in range(2)]
                        last_kb = qe // P - 1

                        def emit_pv(kb, ex, hh):
                            qlo = kb * P
                            for qt in range(qs // QT, qe // QT):
                                rs, re = qt * QT, (qt + 1) * QT
                                if re <= qlo:
                                    continue
                                cs = max(qlo, rs)
                                nc.tensor.matmul(
                                    ps_o[hh][:, cs - qs:re - qs],
                                    Vt[:, kb, 2 * pr + hh, :],
                                    ex[:, hh, cs - qs:re - qs],
                                    start=(kb == 0),
                                    stop=(kb == min(last_kb, re // P - 1)),
                                )

                        prev = None  # (kb, ex) with PV not yet emitted
                        for kb in range(qe // P):
                            qlo = kb * P
                            ex = expp.tile([P, 2, HALF], BF16, tag="ex",
                                           name=f"ex_{pr}_{qh}_{kb}")
                            vstart = max(qlo, qs)
                            for hh in range(2):
                                s_ps = psS.tile([P, HALF], F32, tag=f"s{hh}",
                                                name=f"s{hh}_{pr}_{qh}_{kb}")
                                po = hh * DIM_HEAD
                                for qt in range(qs // QT, qe // QT):
                                    rs, re = qt * QT, (qt + 1) * QT
                                    if re <= qlo:
                                        continue
                                    cs = max(qlo, rs)
                                    nc.tensor.matmul(
                                        s_ps[:, cs - qs:re - qs],
                                        KTt[pr][po:po + DIM_HEAD, qlo:qlo + P],
                                        QTt[pr][po:po + DIM_HEAD, cs:re],
                                        start=True, stop=True,
                                        tile_position=(po, 0),
                                    )
                                nc.scalar.activation(
                                    ex[:, hh, vstart - qs:],
                                    s_ps[:, vstart - qs:],
                                    AF.Exp,
                                )
                                if qlo >= qs:
                                    nc.vector.tensor_tensor(
                                        ex[:, hh, qlo - qs:qlo - qs + P],
                                        ex[:, hh, qlo - qs:qlo - qs + P],
                                        tri[:],
                                        ALU.mult,
                                    )
                                if prev is not None:
                                    emit_pv(prev[0], prev[1], hh)
                            prev = (kb, ex)
                        for hh in range(2):
                            emit_pv(prev[0], prev[1], hh)

                        # evacuate unnormalized output + denom rows quickly
                        # (releases the PV psum banks for the next head pair),
                        # then normalize outT in place: reciprocal runs in a
                        # [128, 16] layout via DRAM shuffles (partition-1
                        # tiles pay full free-size cycles on DVE).
                        da = dramp.tile([2, HALF], F32, tag="da",
                                        name=f"da{pr}_{qh}")
                        for hh in range(2):
                            nc.vector.tensor_copy(
                                outT[pr][qh][hh * DIM_HEAD:(hh + 1) * DIM_HEAD, :],
                                ps_o[hh][:DIM_HEAD, :],
                            )
                            dr = drp.tile([1, HALF], F32, tag=f"dr{hh}",
                                          name=f"dr{pr}_{qh}_{hh}")
                            nc.scalar.copy(
                                dr[:], ps_o[hh][DIM_HEAD:DIM_HEAD + 1, :]
                            )
                            nc.sync.dma_start(da[hh:hh + 1, :], dr[:])
                        dsh = drp.tile([P, 2, HALF // P], F32, tag="dsh",
                                       name=f"dsh{pr}_{qh}")
                        nc.sync.dma_start(
                            dsh[:],
                            da[:].rearrange("h (p o) -> p h o", o=HALF // P),
                        )
                        nc.vector.reciprocal(dsh[:], dsh[:])
                        db = dramp.tile([2, HALF], F32, tag="db",
                                        name=f"db{pr}_{qh}")
                        nc.sync.dma_start(
                            db[:].rearrange("h (p o) -> p h o", o=HALF // P),
                            dsh[:],
                        )
                        rbc = rbcp.tile([P, HALF], F32, tag="rbc",
                                        name=f"rbc{pr}_{qh}")
                        for hh in range(2):
                            nc.sync.dma_start(
                                rbc[hh * DIM_HEAD:(hh + 1) * DIM_HEAD, :],
                                db[hh:hh + 1, :].to_broadcast((DIM_HEAD, HALF)),
                            )
                        nc.vector.tensor_tensor(
                            outT[pr][qh][:], outT[pr][qh][:], rbc[:], ALU.mult
                        )

            # ---- phase C: out projection ----
            psP = ctx.enter_context(tc.tile_pool(name="psP", bufs=3, space="PSUM"))
            for qb in range(NB):
                ps = psP.tile([P, 2, 512], F32, tag="pp")
                for nt in range(2):
                    for pb in range(2):
                        nc.tensor.matmul(
                            ps[:, nt, :],
                            outT[pb][qb // 8][:, (qb % 8) * P:(qb % 8 + 1) * P],
                            wo_bf[:, pb, nt * 512:(nt + 1) * 512],
                            start=(pb == 0), stop=(pb == 1),
                        )
                so = stage.tile([P, DIM], F32, tag="so")
                if qb % 2 == 0:
                    nc.scalar.copy(so[:], ps[:].rearrange("p a f -> p (a f)"))
                else:
                    nc.vector.tensor_copy(so[:], ps[:].rearrange("p a f -> p (a f)"))
                nc.sync.dma_start(out_d[qb * P:(qb + 1) * P, :], so[:])

    nc.compile()
    return nc


def make_in_maps(x, ln_w, ln_b, w_qkv, w_out):
    x = np.asarray(x, np.float32)
    ln_w = np.asarray(ln_w, np.float32)
    ln_b = np.asarray(ln_b, np.float32)
    w_qkv = np.asarray(w_qkv, np.float32)
    w_out = np.asarray(w_out, np.float32)
    bf16 = ml_dtypes.bfloat16

    def perm_w(w):
        # device row (p, kb) holds dim d = kb*P + p — both for the XBAR DMA
        # transpose (verified on hw) and the PE-transpose fallback
        return np.ascontiguousarray(
            w.reshape(KB, P, -1).transpose(1, 0, 2)).astype(bf16)

    in_maps = []
    for c in range(N_CORES):
        b, g = c // 4, c % 4
        cols = np.arange(4 * g * DIM_HEAD, (4 * g + H_LOC) * DIM_HEAD)
        wq_s = w_qkv[:, cols]
        wk_s = w_qkv[:, INNER + cols]
        wv_s = w_qkv[:, 2 * INNER + cols]
        wq = perm_w(ln_w[:, None] * wq_s * SCALE)
        wk = perm_w(ln_w[:, None] * wk_s)
        wv = perm_w(ln_w[:, None] * wv_s)
        wo = np.ascontiguousarray(
            w_out[cols, :].reshape(2, P, DIM).transpose(1, 0, 2)).astype(bf16)
        bq = (ln_b @ wq_s) * SCALE
        bk = ln_b @ wk_s
        bv = ln_b @ wv_s
        in_maps.append({
            "x": np.ascontiguousarray(x[b]),
            "wq": wq, "wk": wk, "wv": wv, "wo": wo,
            "bq": np.ascontiguousarray(bq.reshape(2, P).T),
            "bk": np.ascontiguousarray(bk.reshape(2, P).T),
            "bv": bv.reshape(1, H_LOC * DIM_HEAD),
        })
    return in_maps


_NC_CACHE = []


def kernel(x, ln_w, ln_b, w_qkv, w_out):
    in_maps = make_in_maps(x, ln_w, ln_b, w_qkv, w_out)
    if not _NC_CACHE:
        _NC_CACHE.append(build_nc())
    nc = _NC_CACHE[0]
    res = run_bass_kernel_spmd(nc, in_maps, list(range(N_CORES))).results
    out = np.zeros((B, N, DIM), np.float32)
    for c in range(N_CORES):
        out[c // 4] += res[c]["out"]
    return out
